# revision 5
# baseline (speedup 1.0000x reference)
"""Trainium2 Bass kernel for nn_BoundaryLoss (Sobel-boundary BCE loss).

loss = mean(softplus(z) - z*et) over B=64 images of 512x512, where
  ps  = sigmoid(p)
  z   = |conv(ps,GX)| + |conv(ps,GY)|          (SAME zero padding)
  et  = ((|conv(t,GX)| + |conv(t,GY)|) > 0)    (t binary)

Device strategy (8 cores, pure data parallel over batch):
  * p and t shipped as fp8e4m3 (t exact for 0/1); sigmoid output ps also
    fp8 so every conv matmul runs in fp8 DoubleRow mode (0.5 cyc/row).
  * |ex|+|ey| = max(|u|,|v|) with u = conv(ps, GX+GY), v = conv(ps, GX-GY)
    -- one abs_max tensor-tensor op straight out of PSUM replaces the
    abs-transit + add of the naive form.
  * Each of the three convs (u, v, wt = conv(t, GX+9GY)) is 2 DoubleRow
    matmuls: taps (j0,j1) paired, (j2, zero-row) paired.
  * Sign-folded softplus: loss_elem = -ln sigmoid(w), w = +z if et else -z.
    et = (wt != 0); the fold is pure bit math: m = (wt==0)<<15 (u16),
    w = z | m on the bf16 zstore.
  * Phase 2: sigmoid(w) -> 8-ary bf16 product tree -> one Ln pass with
    accum_out.  Exactly 2 ACT table loads (Sigmoid warm at t=0, Ln at end).
  * H split into 4 bands of 126 rows + one packed band (last 8 rows of all
    8 images block-diagonally, 72 partitions) -> no halo corrections.
  * DMA batched per image-pair (4 transfers per pair) to amortize the
    625ns/op HWDGE serialization; banded fp8 weight matrices shipped
    without their zero rows (memset on device).
  * Engines: PE 6 matmuls/unit; DVE abs_max + fold + product tree;
    Pool (gpsimd) the (wt==0) mask + 1/3 of abs_max; ACT sigmoids + Ln.
  * Device covers image cols 1..511 (+ phantom col 512 from the zero pad);
    host adds the w=0 column, subtracts the phantom contribution, and
    subtracts softplus(0)=ln2 for the structurally-zero junk rows.
"""

import os
import sys

import numpy as np

for _p in ("/opt/trn_rl_repo", os.path.expanduser("~/.axon_site/_ro/trn_rl_repo")):
    if os.path.isdir(_p) and _p not in sys.path:
        sys.path.append(_p)

import concourse.bass as bass
import concourse.bacc as bacc
import concourse.tile as tile
from concourse import mybir
from concourse.bass import _add_dep_helper
from concourse.bass_utils import run_bass_kernel_spmd

F32 = mybir.dt.float32
BF16 = mybir.dt.bfloat16
U32 = mybir.dt.uint32
U16 = mybir.dt.uint16
FP8 = mybir.dt.float8e4
U8 = mybir.dt.uint8
A = mybir.AluOpType
AF = mybir.ActivationFunctionType

NCORES = 8
B, H, W = 64, 512, 512
BPC = B // NCORES          # images per core
NB = 4                     # full 126-row bands per image
BAND = 126
NBP = BPC * NB + 1         # units per core (33)
WP = W + 4                 # padded tile width for shifted DoubleRow reads

# 3x3 kernels and the three vertical-tap sets
_GX = np.array([[1., 0., -1.], [2., 0., -2.], [1., 0., -1.]])
_GY = np.array([[1., 2., 1.], [0., 0., 0.], [-1., -2., -1.]])
_CU = _GX + _GY            # u-conv:  |ex|+|ey| = max(|u|,|v|)
_CV = _GX - _GY
_CW = _GX + 9.0 * _GY      # wt-conv: et = (wt != 0), exact ints in fp8
_CONVS = [_CU, _CV, _CW]


def _banded(tap, var):
    """[128,128] banded vertical-conv matrix for one tap and variant."""
    m = np.zeros((128, 128), np.float32)
    if var < 2:
        for q in range(BAND):
            for dh in range(3):
                p = q + dh - (1 if var == 1 else 0)
                if 0 <= p < 128:
                    m[p, q] = tap[dh]
    else:
        for j in range(BPC):
            for qq in range(8):
                for dh in range(3):
                    pp = qq + dh
                    if pp < 9:
                        m[9 * j + pp, 8 * j + qq] = tap[dh]
    return m


def _banded_mats():
    """[128, 27, 128] fp8: var(3) x conv(3) x tap(3) banded lhsT rows."""
    out = np.zeros((128, 27, 128), np.float32)
    for var in range(3):
        for ci, cm in enumerate(_CONVS):
            for k in range(3):
                out[:, var * 9 + ci * 3 + k, :] = _banded(cm[:, k], var)
    return out.astype(mybir.dt.np(FP8))


def _build_program(opts=()):
    opts = set(opts)
    nc = bacc.Bacc("TRN2", target_bir_lowering=False)
    p_d = nc.dram_tensor("p", [BPC, H, W], FP8, kind="ExternalInput")
    t_d = nc.dram_tensor("t", [BPC, H, W], FP8, kind="ExternalInput")
    bk_d = nc.dram_tensor("bk", [128, 27, 128], FP8, kind="ExternalInput")
    out_d = nc.dram_tensor("out", [128, 2], F32, kind="ExternalOutput")
    if "debug" in opts:
        dbg_z = nc.dram_tensor("dbg_z", [128, NBP, W], BF16, kind="ExternalOutput")
        dbg_q = nc.dram_tensor("dbg_q", [128, NBP, W], BF16, kind="ExternalOutput")

    DR = mybir.MatmulPerfMode.DoubleRow

    with tile.TileContext(nc) as tc:
        with tc.tile_pool(name="consts", bufs=1) as consts, \
             tc.tile_pool(name="xin", bufs=1) as xin, \
             tc.tile_pool(name="tin", bufs=1) as tin, \
             tc.tile_pool(name="psg", bufs=1) as psg, \
             tc.tile_pool(name="packed", bufs=1) as packed, \
             tc.tile_pool(name="mm", bufs=2) as mmp, \
             tc.tile_pool(name="ax", bufs=2) as axp, \
             tc.tile_pool(name="zs", bufs=1) as zs, \
             tc.tile_pool(name="accp", bufs=1) as accp, \
             tc.tile_pool(name="psum", bufs=2, space="PSUM") as psum, \
             tc.tile_pool(name="psum2", bufs=2, space="PSUM") as psum2:

            bk = consts.tile([128, 36, 128], FP8)   # var x conv x (t0,t1,t2,Z)

            warm = accp.tile([1, 1], F32)
            nc.gpsimd.memset(warm[:, :], 0.0)
            nc.scalar.activation(out=warm[:, :], in_=warm[:, :],
                                 func=AF.Sigmoid)

            x_all = xin.tile([128, BPC, NB, W], FP8)
            t_all = tin.tile([128, BPC, NB, WP], FP8)
            ps_all = psg.tile([128, BPC, NB, WP], FP8)
            x4 = packed.tile([72, W], FP8)
            t4 = packed.tile([72, WP], FP8)
            ps4 = packed.tile([72, WP], FP8)
            zstore = zs.tile([128, NBP, W], BF16)
            sq = zs.tile([128, NBP, W], BF16)
            acc_s = accp.tile([128, 1], F32)
            out_t = accp.tile([128, 2], F32)

            # zero right-pads (read by shifted DoubleRow taps), Z weight
            # rows, and the output accumulators
            nc.gpsimd.memset(t_all[:, :, :, W:WP].bitcast(U32), 0)
            nc.gpsimd.memset(ps_all[:, :, :, W:WP].bitcast(U32), 0)
            nc.gpsimd.memset(t4[:, W:WP].bitcast(U32), 0)
            nc.gpsimd.memset(ps4[:, W:WP].bitcast(U32), 0)
            nc.gpsimd.memset(bk[:, 3:36:4, :].bitcast(U8), 0)
            nc.gpsimd.memset(out_t[:, :], 0.0)

            # ---- input DMA: one transfer for all band-0 slabs, one per
            # image for bands 1-3 (DMA APs are limited to 3 dims) ----
            def b0_dma(dram, dst):
                nc.sync.dma_start(
                    out=dst[:, :, 0, 0:W],
                    in_=bass.AP(tensor=dram[:, :, :].tensor, offset=0,
                                ap=[[W, 128], [H * W, BPC], [1, W]]))

            def b123_dma(dram, dst, i):
                nc.sync.dma_start(
                    out=dst[:, i, 1:4, 0:W],
                    in_=bass.AP(tensor=dram[:, :, :].tensor,
                                offset=i * H * W + (BAND - 1) * W,
                                ap=[[W, 128], [BAND * W, 3], [1, W]]))

            b0_dma(p_d, x_all)
            b123_dma(p_d, x_all, 0)
            b123_dma(p_d, x_all, 1)
            # banded weights (tap rows only; Z rows memset above)
            nc.sync.dma_start(
                out=bass.AP(tensor=bk.tensor, offset=bk.offset,
                            ap=[[bk.ap[0][0], 128], [4 * 128, 9],
                                [128, 3], [1, 128]]),
                in_=bass.AP(tensor=bk_d[:, :, :].tensor, offset=0,
                            ap=[[27 * 128, 128], [3 * 128, 9],
                                [128, 3], [1, 128]]))
            b0_dma(t_d, t_all)
            b123_dma(t_d, t_all, 0)
            b123_dma(t_d, t_all, 1)
            for i in range(2, BPC):
                b123_dma(p_d, x_all, i)
                b123_dma(t_d, t_all, i)
            # packed band: last 9 rows of each image, partitions 9j+q = 0..71
            src9 = lambda dram: bass.AP(
                tensor=dram[:, :, :].tensor, offset=(H - 9) * W,
                ap=[[H * W, 8], [W, 9], [1, W]])
            nc.sync.dma_start(out=x4[0:72, :], in_=src9(p_d))
            nc.sync.dma_start(out=t4[0:72, 0:W], in_=src9(t_d))

            # ---- unit list ----
            units = []      # (vb, kk, ps_view, t_view)
            for i in range(BPC):
                for b in range(NB):
                    vb = 1 if b == 0 else 0
                    units.append((vb, 128, ps_all[:, i, b, :], t_all[:, i, b, :]))
            units.append((2, 72, ps4[:, :], t4[:, :]))

            def drmm(outap, kk, row, rhs_view, col, start, stop):
                rhs = bass.AP(tensor=rhs_view.tensor,
                              offset=rhs_view.offset + col,
                              ap=[[rhs_view.ap[0][0], kk], [1, 2], [1, W]])
                nc.tensor.matmul(outap, bk[0:kk, row:row + 2, :], rhs,
                                 start=start, stop=stop, perf_mode=DR)

            AX = mybir.AxisListType
            sig_w = []
            wt_pair = None
            s1 = None
            for u, (vb, kk, psv, tv) in enumerate(units):
                if u % 8 == 0:
                    # sigmoid for the image pair feeding units u..u+7
                    g = u // 8
                    if g < 4:
                        nc.scalar.activation(
                            out=ps_all[:, 2 * g:2 * g + 2, :, 0:W],
                            in_=x_all[:, 2 * g:2 * g + 2, :, :],
                            func=AF.Sigmoid)
                    else:
                        nc.scalar.activation(out=ps4[:, 0:W], in_=x4[:, :],
                                             func=AF.Sigmoid)

                P = psum.tile([128, 2, W], F32, tag="uv")
                half = u % 2
                if half == 0:
                    wt_pair = psum2.tile([128, 2, W], F32, tag="wt")
                    s1 = mmp.tile([128, 2, W], BF16, tag="s1")
                r0 = vb * 12
                drmm(P[:, 0, :], kk, r0 + 0, psv, 0, True, False)
                drmm(P[:, 0, :], kk, r0 + 2, psv, 2, False, True)
                drmm(P[:, 1, :], kk, r0 + 4, psv, 0, True, False)
                drmm(P[:, 1, :], kk, r0 + 6, psv, 2, False, True)
                drmm(wt_pair[:, half, :], kk, r0 + 8, tv, 0, True, False)
                drmm(wt_pair[:, half, :], kk, r0 + 10, tv, 2, False, True)

                # z = max(|u|, |v|): ~3/7 of units via ACT Abs + gpsimd max,
                # the rest as one DVE abs-max reduce over the (u,v) axis
                if u % 7 < 3:
                    ax = axp.tile([128, 2, W], BF16, tag="ax")
                    nc.scalar.activation(out=ax[:, :, :], in_=P[:, :, :],
                                         func=AF.Abs)
                    nc.gpsimd.tensor_tensor(
                        out=zstore[:, u, :], in0=ax[:, 0, :], in1=ax[:, 1, :],
                        op=A.max)
                else:
                    nc.vector.tensor_reduce(
                        out=zstore[:, u, :],
                        in_=P.rearrange("p c w -> p w c"), axis=AX.X,
                        op=A.max, apply_absolute_value=True)

                if half == 1 or u == NBP - 1:
                    u0 = u - half
                    # s1 = (wt != 0) - 0.5 in {-0.5, +0.5}
                    nc.vector.tensor_scalar(
                        out=s1[:, 0:half + 1, :],
                        in0=wt_pair[:, 0:half + 1, :],
                        scalar1=0.0, scalar2=0.5,
                        op0=A.not_equal, op1=A.subtract)
                    # q' = z * s1 = +-z/2 (sigmoid below uses scale=2)
                    q_eng = nc.vector if (u // 2) % 5 < 3 else nc.gpsimd
                    q_eng.tensor_tensor(
                        out=zstore[:, u0:u + 1, :],
                        in0=zstore[:, u0:u + 1, :],
                        in1=s1[:, 0:half + 1, :], op=A.mult)

                # sigmoid(2*q') in chunks of 4 units
                if (u + 1) % 4 == 0 or u == NBP - 1:
                    lo = u // 4 * 4
                    s2 = nc.scalar.activation(
                        out=sq[:, lo:u + 1, :], in_=zstore[:, lo:u + 1, :],
                        func=AF.Sigmoid, scale=2.0)
                    sig_w.append(s2)

                # 8-ary product tree per completed group of 8
                if (u + 1) % 8 == 0:
                    g0 = u - 7
                    p_eng = nc.vector if (u // 8) % 5 == 0 else nc.gpsimd
                    p_eng.tensor_tensor(
                        out=sq[:, g0:g0 + 8:2, :], in0=sq[:, g0:g0 + 8:2, :],
                        in1=sq[:, g0 + 1:g0 + 8:2, :], op=A.mult)
                    p_eng.tensor_tensor(
                        out=sq[:, g0:g0 + 8:4, :], in0=sq[:, g0:g0 + 8:4, :],
                        in1=sq[:, g0 + 2:g0 + 8:4, :], op=A.mult)
                    p_eng.tensor_tensor(
                        out=sq[:, g0, :], in0=sq[:, g0, :],
                        in1=sq[:, g0 + 4, :], op=A.mult)

            if "debug" in opts:
                nc.sync.dma_start(out=dbg_z[:, :, :], in_=zstore[:, :, :])
                nc.sync.dma_start(out=dbg_q[:, :, :], in_=sq[:, :, :])

            # ---- finale: one Ln pass over the 5 product slots ----
            li = nc.scalar.activation(
                out=zstore[:, 0:NBP:8, :], in_=sq[:, 0:NBP:8, :],
                func=AF.Ln, accum_out=acc_s[:, 0:1])
            _add_dep_helper(li.ins, sig_w[-1].ins, sync=True,
                            reason="ACT table phase split")
            nc.vector.tensor_copy(out=out_t[:, 0:1], in_=acc_s[:, 0:1])
            nc.sync.dma_start(out=out_d[:, :], in_=out_t)

    nc.compile()
    return nc


_NC_CACHE = None


def _get_nc():
    global _NC_CACHE
    if _NC_CACHE is None:
        _NC_CACHE = _build_program()
    return _NC_CACHE


def _edge_loss_sum(p, t):
    """float64 loss sum over the w=0 column (not computed on device)."""
    ps = 1.0 / (1.0 + np.exp(-p.astype(np.float64)))
    td = t.astype(np.float64)

    def slab(x):
        s = np.zeros((B, H + 2, 3))
        s[:, 1:H + 1, 1:3] = x[:, :, 0:2]
        return s

    sp, st = slab(ps), slab(td)

    def conv(x, K):
        acc = np.zeros((B, H))
        for dh in range(3):
            for dw in range(3):
                acc += K[dh, dw] * x[:, dh:dh + H, dw]
        return acc

    z = np.abs(conv(sp, _GX)) + np.abs(conv(sp, _GY))
    et = (np.abs(conv(st, _GX)) + np.abs(conv(st, _GY))) > 0
    return (np.logaddexp(0.0, z) - z * et).sum()


def _phantom_loss_sum(p, t):
    """float64 loss sum the device adds for its phantom column (image col
    512, fed by image col 511 + zero pads); subtracted from the total."""
    ps_col = 1.0 / (1.0 + np.exp(-p[:, :, W - 1].astype(np.float64)))
    t_col = t[:, :, W - 1].astype(np.float64)

    def vconv(col, tap):
        s = np.zeros((B, H + 2))
        s[:, 1:H + 1] = col
        return tap[0] * s[:, 0:H] + tap[1] * s[:, 1:H + 1] + tap[2] * s[:, 2:H + 2]

    ex = vconv(ps_col, np.array([1.0, 2.0, 1.0]))
    ey = vconv(ps_col, np.array([1.0, 0.0, -1.0]))
    wt = vconv(t_col, np.array([10.0, 2.0, -8.0]))
    z = np.abs(ex) + np.abs(ey)
    et = wt != 0
    return (np.logaddexp(0.0, z) - z * et).sum()


def kernel(p: np.ndarray, t: np.ndarray) -> np.ndarray:
    p = np.ascontiguousarray(np.asarray(p, dtype=np.float32)).reshape(B, H, W)
    t = np.ascontiguousarray(np.asarray(t, dtype=np.float32)).reshape(B, H, W)
    nc = _get_nc()
    bk = _banded_mats()
    p8 = p.astype(mybir.dt.np(FP8))
    t8 = t.astype(mybir.dt.np(FP8))
    in_maps = [
        {"p": p8[c * BPC:(c + 1) * BPC], "t": t8[c * BPC:(c + 1) * BPC],
         "bk": bk}
        for c in range(NCORES)
    ]
    res = run_bass_kernel_spmd(nc, in_maps, core_ids=list(range(NCORES)))
    # junk rows carried by the 128-partition tiles: 2 zero rows in each of
    # the 32 full band tiles + 64 zero rows in the packed tile; each
    # contributes softplus(0) = ln 2 at W columns.
    junk = (2 * BPC * NB + (128 - 8 * BPC)) * W * np.log(2.0)
    total = 0.0
    for c in range(NCORES):
        o = res.results[c]["out"].astype(np.float64)
        total += -o[:, 0].sum() - junk
    total += _edge_loss_sum(p, t) - _phantom_loss_sum(p, t)
    return np.float32(total / (B * H * W))


# revision 8
# speedup vs baseline: 1.0062x; 1.0062x over previous
"""Trainium2 Bass kernel for nn_BoundaryLoss (Sobel-boundary BCE loss).

loss = mean(softplus(z) - z*et) over B=64 images of 512x512, where
  ps  = sigmoid(p)
  z   = |conv(ps,GX)| + |conv(ps,GY)|          (SAME zero padding)
  et  = ((|conv(t,GX)| + |conv(t,GY)|) > 0)    (t binary)

Device strategy (8 cores, pure data parallel over batch):
  * p and t shipped as fp8e4m3 (t exact for 0/1); sigmoid output ps also
    fp8 so every conv matmul runs in fp8 DoubleRow mode (0.5 cyc/row).
  * |ex|+|ey| = max(|u|,|v|) with u = conv(ps, GX+GY), v = conv(ps, GX-GY)
    -- one abs_max tensor-tensor op straight out of PSUM replaces the
    abs-transit + add of the naive form.
  * Each of the three convs (u, v, wt = conv(t, GX+9GY)) is 2 DoubleRow
    matmuls: taps (j0,j1) paired, (j2, zero-row) paired.
  * Sign-folded softplus: loss_elem = -ln sigmoid(w), w = +z if et else -z.
    et = (wt != 0); the fold is pure bit math: m = (wt==0)<<15 (u16),
    w = z | m on the bf16 zstore.
  * Phase 2: sigmoid(w) -> 8-ary bf16 product tree -> one Ln pass with
    accum_out.  Exactly 2 ACT table loads (Sigmoid warm at t=0, Ln at end).
  * H split into 4 bands of 126 rows + one packed band (last 8 rows of all
    8 images block-diagonally, 72 partitions) -> no halo corrections.
  * DMA batched per image-pair (4 transfers per pair) to amortize the
    625ns/op HWDGE serialization; banded fp8 weight matrices shipped
    without their zero rows (memset on device).
  * Engines: PE 6 matmuls/unit; DVE abs_max + fold + product tree;
    Pool (gpsimd) the (wt==0) mask + 1/3 of abs_max; ACT sigmoids + Ln.
  * Device covers image cols 1..511 (+ phantom col 512 from the zero pad);
    host adds the w=0 column, subtracts the phantom contribution, and
    subtracts softplus(0)=ln2 for the structurally-zero junk rows.
"""

import os
import sys

import numpy as np

for _p in ("/opt/trn_rl_repo", os.path.expanduser("~/.axon_site/_ro/trn_rl_repo")):
    if os.path.isdir(_p) and _p not in sys.path:
        sys.path.append(_p)

import concourse.bass as bass
import concourse.bacc as bacc
import concourse.tile as tile
from concourse import mybir
from concourse.bass import _add_dep_helper
from concourse.bass_utils import run_bass_kernel_spmd

F32 = mybir.dt.float32
BF16 = mybir.dt.bfloat16
U32 = mybir.dt.uint32
U16 = mybir.dt.uint16
FP8 = mybir.dt.float8e4
U8 = mybir.dt.uint8
A = mybir.AluOpType
AF = mybir.ActivationFunctionType

NCORES = 8
B, H, W = 64, 512, 512
BPC = B // NCORES          # images per core
NB = 4                     # full 126-row bands per image
BAND = 126
NBP = BPC * NB + 1         # units per core (33)
WP = W + 4                 # padded tile width for shifted DoubleRow reads

# 3x3 kernels and the vertical-tap sets
_GX = np.array([[1., 0., -1.], [2., 0., -2.], [1., 0., -1.]])
_GY = np.array([[1., 2., 1.], [0., 0., 0.], [-1., -2., -1.]])
_CU = _GX + _GY            # u-conv:  |ex|+|ey| = max(|u|,|v|)
_CV = _GX - _GY
_CW = _GX + 9.0 * _GY      # wt-conv: et = (wt != 0), exact ints in fp8
# DVE-path units conv (U, V) and reduce with abs; ACT-path units conv
# (GX, GY) so the combine after ACT-Abs is an ADD (the only elementwise
# reduction Pool's ISA has).
_CONVS = [_CU, _CV, _CW, _GX, _GY]
NCV = len(_CONVS)


def _banded(tap, var):
    """[128,128] banded vertical-conv matrix for one tap and variant."""
    m = np.zeros((128, 128), np.float32)
    if var < 2:
        for q in range(BAND):
            for dh in range(3):
                p = q + dh - (1 if var == 1 else 0)
                if 0 <= p < 128:
                    m[p, q] = tap[dh]
    else:
        for j in range(BPC):
            for qq in range(8):
                for dh in range(3):
                    pp = qq + dh
                    if pp < 9:
                        m[9 * j + pp, 8 * j + qq] = tap[dh]
    return m


def _banded_mats():
    """[128, 3*NCV*3, 128] fp8: var(3) x conv(NCV) x tap(3) banded lhsT."""
    out = np.zeros((128, 3 * NCV * 3, 128), np.float32)
    for var in range(3):
        for ci, cm in enumerate(_CONVS):
            for k in range(3):
                out[:, (var * NCV + ci) * 3 + k, :] = _banded(cm[:, k], var)
    return out.astype(mybir.dt.np(FP8))


def _build_program(opts=()):
    opts = set(opts)
    nc = bacc.Bacc("TRN2", target_bir_lowering=False)
    p_d = nc.dram_tensor("p", [BPC, H, W], FP8, kind="ExternalInput")
    t_d = nc.dram_tensor("t", [BPC, H, W], FP8, kind="ExternalInput")
    bk_d = nc.dram_tensor("bk", [128, 3 * NCV * 3, 128], FP8, kind="ExternalInput")
    out_d = nc.dram_tensor("out", [128, 2], F32, kind="ExternalOutput")
    if "debug" in opts:
        dbg_z = nc.dram_tensor("dbg_z", [128, NBP, W], BF16, kind="ExternalOutput")
        dbg_q = nc.dram_tensor("dbg_q", [128, NBP, W], BF16, kind="ExternalOutput")

    DR = mybir.MatmulPerfMode.DoubleRow

    with tile.TileContext(nc) as tc:
        with tc.tile_pool(name="consts", bufs=1) as consts, \
             tc.tile_pool(name="xin", bufs=1) as xin, \
             tc.tile_pool(name="tin", bufs=1) as tin, \
             tc.tile_pool(name="psg", bufs=1) as psg, \
             tc.tile_pool(name="packed", bufs=1) as packed, \
             tc.tile_pool(name="mm", bufs=2) as mmp, \
             tc.tile_pool(name="ax", bufs=2) as axp, \
             tc.tile_pool(name="zs", bufs=1) as zs, \
             tc.tile_pool(name="accp", bufs=1) as accp, \
             tc.tile_pool(name="psum", bufs=2, space="PSUM") as psum, \
             tc.tile_pool(name="psum2", bufs=2, space="PSUM") as psum2:

            bk = consts.tile([128, 3 * NCV * 4, 128], FP8)  # var x conv x (t0,t1,t2,Z)

            warm = accp.tile([1, 1], F32)
            nc.gpsimd.memset(warm[:, :], 0.0)
            nc.scalar.activation(out=warm[:, :], in_=warm[:, :],
                                 func=AF.Sigmoid)

            x_all = xin.tile([128, BPC, NB, W], FP8)
            t_all = tin.tile([128, BPC, NB, WP], FP8)
            ps_all = psg.tile([128, BPC, NB, WP], FP8)
            x4 = packed.tile([72, W], FP8)
            t4 = packed.tile([72, WP], FP8)
            ps4 = packed.tile([72, WP], FP8)
            zstore = zs.tile([128, NBP, W], BF16)
            sq = zs.tile([128, NBP, W], BF16)
            acc_s = accp.tile([128, 1], F32)
            out_t = accp.tile([128, 2], F32)

            # zero right-pads (read by shifted DoubleRow taps), Z weight
            # rows, and the output accumulators
            nc.gpsimd.memset(t_all[:, :, :, W:WP].bitcast(U32), 0)
            nc.gpsimd.memset(ps_all[:, :, :, W:WP].bitcast(U32), 0)
            nc.gpsimd.memset(t4[:, W:WP].bitcast(U32), 0)
            nc.gpsimd.memset(ps4[:, W:WP].bitcast(U32), 0)
            nc.gpsimd.memset(bk[:, 3:3 * NCV * 4:4, :].bitcast(U8), 0)
            nc.gpsimd.memset(out_t[:, :], 0.0)

            # ---- input DMA: one transfer for all band-0 slabs, one per
            # image for bands 1-3 (DMA APs are limited to 3 dims) ----
            def b0_dma(dram, dst):
                nc.sync.dma_start(
                    out=dst[:, :, 0, 0:W],
                    in_=bass.AP(tensor=dram[:, :, :].tensor, offset=0,
                                ap=[[W, 128], [H * W, BPC], [1, W]]))

            def b123_dma(dram, dst, i):
                nc.sync.dma_start(
                    out=dst[:, i, 1:4, 0:W],
                    in_=bass.AP(tensor=dram[:, :, :].tensor,
                                offset=i * H * W + (BAND - 1) * W,
                                ap=[[W, 128], [BAND * W, 3], [1, W]]))

            b0_dma(p_d, x_all)
            b123_dma(p_d, x_all, 0)
            b123_dma(p_d, x_all, 1)
            # banded weights (tap rows only; Z rows memset above)
            nc.sync.dma_start(
                out=bass.AP(tensor=bk.tensor, offset=bk.offset,
                            ap=[[bk.ap[0][0], 128], [4 * 128, 3 * NCV],
                                [128, 3], [1, 128]]),
                in_=bass.AP(tensor=bk_d[:, :, :].tensor, offset=0,
                            ap=[[3 * NCV * 3 * 128, 128],
                                [3 * 128, 3 * NCV],
                                [128, 3], [1, 128]]))
            b0_dma(t_d, t_all)
            b123_dma(t_d, t_all, 0)
            b123_dma(t_d, t_all, 1)
            for i in range(2, BPC):
                b123_dma(p_d, x_all, i)
                b123_dma(t_d, t_all, i)
            # packed band: last 9 rows of each image, partitions 9j+q = 0..71
            src9 = lambda dram: bass.AP(
                tensor=dram[:, :, :].tensor, offset=(H - 9) * W,
                ap=[[H * W, 8], [W, 9], [1, W]])
            nc.sync.dma_start(out=x4[0:72, :], in_=src9(p_d))
            nc.sync.dma_start(out=t4[0:72, 0:W], in_=src9(t_d))

            # ---- unit list ----
            units = []      # (vb, kk, ps_view, t_view)
            for i in range(BPC):
                for b in range(NB):
                    vb = 1 if b == 0 else 0
                    units.append((vb, 128, ps_all[:, i, b, :], t_all[:, i, b, :]))
            units.append((2, 72, ps4[:, :], t4[:, :]))

            def drmm(outap, kk, row, rhs_view, col, start, stop):
                rhs = bass.AP(tensor=rhs_view.tensor,
                              offset=rhs_view.offset + col,
                              ap=[[rhs_view.ap[0][0], kk], [1, 2], [1, W]])
                nc.tensor.matmul(outap, bk[0:kk, row:row + 2, :], rhs,
                                 start=start, stop=stop, perf_mode=DR)

            AX = mybir.AxisListType
            sig_w = []
            wt_pair = None
            s1 = None
            for u, (vb, kk, psv, tv) in enumerate(units):
                if u % 8 == 0:
                    # sigmoid for the image pair feeding units u..u+7
                    g = u // 8
                    if g < 4:
                        nc.scalar.activation(
                            out=ps_all[:, 2 * g:2 * g + 2, :, 0:W],
                            in_=x_all[:, 2 * g:2 * g + 2, :, :],
                            func=AF.Sigmoid)
                    else:
                        nc.scalar.activation(out=ps4[:, 0:W], in_=x4[:, :],
                                             func=AF.Sigmoid)

                P = psum.tile([128, 2, W], F32, tag="uv")
                half = u % 2
                if half == 0:
                    wt_pair = psum2.tile([128, 2, W], F32, tag="wt")
                    s1 = mmp.tile([128, 2, W], BF16, tag="s1")
                act_path = u % 9 < 4
                base = vb * NCV * 4
                c0 = base + (12 if act_path else 0)   # GX/GY or U/V rows
                drmm(P[:, 0, :], kk, c0 + 0, psv, 0, True, False)
                drmm(P[:, 0, :], kk, c0 + 2, psv, 2, False, True)
                drmm(P[:, 1, :], kk, c0 + 4, psv, 0, True, False)
                drmm(P[:, 1, :], kk, c0 + 6, psv, 2, False, True)
                drmm(wt_pair[:, half, :], kk, base + 8, tv, 0, True, False)
                drmm(wt_pair[:, half, :], kk, base + 10, tv, 2, False, True)

                # z = |ex|+|ey|: ~4/9 of units via ACT Abs(ex,ey) + DVE add;
                # the rest conv (u,v) and z = max(|u|,|v|) as one DVE
                # abs-max reduce over the (u,v) axis
                if act_path:
                    ax = axp.tile([128, 2, W], BF16, tag="ax")
                    nc.scalar.activation(out=ax[:, :, :], in_=P[:, :, :],
                                         func=AF.Abs)
                    nc.vector.tensor_tensor(
                        out=zstore[:, u, :], in0=ax[:, 0, :], in1=ax[:, 1, :],
                        op=A.add)
                else:
                    nc.vector.tensor_reduce(
                        out=zstore[:, u, :],
                        in_=P.rearrange("p c w -> p w c"), axis=AX.X,
                        op=A.max, apply_absolute_value=True)

                if half == 1 or u == NBP - 1:
                    u0 = u - half
                    # s1 = (wt != 0) - 0.5 in {-0.5, +0.5}
                    nc.vector.tensor_scalar(
                        out=s1[:, 0:half + 1, :],
                        in0=wt_pair[:, 0:half + 1, :],
                        scalar1=0.0, scalar2=0.5,
                        op0=A.not_equal, op1=A.subtract)
                    # q' = z * s1 = +-z/2 (sigmoid below uses scale=2)
                    q_eng = nc.vector if (u // 2) % 9 < 4 else nc.gpsimd
                    q_eng.tensor_tensor(
                        out=zstore[:, u0:u + 1, :],
                        in0=zstore[:, u0:u + 1, :],
                        in1=s1[:, 0:half + 1, :], op=A.mult)

                # sigmoid(2*q') in chunks of 4 units
                if (u + 1) % 4 == 0 or u == NBP - 1:
                    lo = u // 4 * 4
                    s2 = nc.scalar.activation(
                        out=sq[:, lo:u + 1, :], in_=zstore[:, lo:u + 1, :],
                        func=AF.Sigmoid, scale=2.0)
                    sig_w.append(s2)

                # 8-ary product tree per completed group of 8
                if (u + 1) % 8 == 0:
                    g0 = u - 7
                    p_eng = nc.gpsimd
                    p_eng.tensor_tensor(
                        out=sq[:, g0:g0 + 8:2, :], in0=sq[:, g0:g0 + 8:2, :],
                        in1=sq[:, g0 + 1:g0 + 8:2, :], op=A.mult)
                    p_eng.tensor_tensor(
                        out=sq[:, g0:g0 + 8:4, :], in0=sq[:, g0:g0 + 8:4, :],
                        in1=sq[:, g0 + 2:g0 + 8:4, :], op=A.mult)
                    p_eng.tensor_tensor(
                        out=sq[:, g0, :], in0=sq[:, g0, :],
                        in1=sq[:, g0 + 4, :], op=A.mult)

            if "debug" in opts:
                nc.sync.dma_start(out=dbg_z[:, :, :], in_=zstore[:, :, :])
                nc.sync.dma_start(out=dbg_q[:, :, :], in_=sq[:, :, :])

            # ---- finale: one Ln pass over the 5 product slots ----
            li = nc.scalar.activation(
                out=zstore[:, 0:NBP:8, :], in_=sq[:, 0:NBP:8, :],
                func=AF.Ln, accum_out=acc_s[:, 0:1])
            _add_dep_helper(li.ins, sig_w[-1].ins, sync=True,
                            reason="ACT table phase split")
            nc.vector.tensor_copy(out=out_t[:, 0:1], in_=acc_s[:, 0:1])
            nc.sync.dma_start(out=out_d[:, :], in_=out_t)

    nc.compile()
    return nc


_NC_CACHE = None


def _get_nc():
    global _NC_CACHE
    if _NC_CACHE is None:
        _NC_CACHE = _build_program()
    return _NC_CACHE


def _edge_loss_sum(p, t):
    """float64 loss sum over the w=0 column (not computed on device)."""
    ps = 1.0 / (1.0 + np.exp(-p.astype(np.float64)))
    td = t.astype(np.float64)

    def slab(x):
        s = np.zeros((B, H + 2, 3))
        s[:, 1:H + 1, 1:3] = x[:, :, 0:2]
        return s

    sp, st = slab(ps), slab(td)

    def conv(x, K):
        acc = np.zeros((B, H))
        for dh in range(3):
            for dw in range(3):
                acc += K[dh, dw] * x[:, dh:dh + H, dw]
        return acc

    z = np.abs(conv(sp, _GX)) + np.abs(conv(sp, _GY))
    et = (np.abs(conv(st, _GX)) + np.abs(conv(st, _GY))) > 0
    return (np.logaddexp(0.0, z) - z * et).sum()


def _phantom_loss_sum(p, t):
    """float64 loss sum the device adds for its phantom column (image col
    512, fed by image col 511 + zero pads); subtracted from the total."""
    ps_col = 1.0 / (1.0 + np.exp(-p[:, :, W - 1].astype(np.float64)))
    t_col = t[:, :, W - 1].astype(np.float64)

    def vconv(col, tap):
        s = np.zeros((B, H + 2))
        s[:, 1:H + 1] = col
        return tap[0] * s[:, 0:H] + tap[1] * s[:, 1:H + 1] + tap[2] * s[:, 2:H + 2]

    ex = vconv(ps_col, np.array([1.0, 2.0, 1.0]))
    ey = vconv(ps_col, np.array([1.0, 0.0, -1.0]))
    wt = vconv(t_col, np.array([10.0, 2.0, -8.0]))
    z = np.abs(ex) + np.abs(ey)
    et = wt != 0
    return (np.logaddexp(0.0, z) - z * et).sum()


def kernel(p: np.ndarray, t: np.ndarray) -> np.ndarray:
    p = np.ascontiguousarray(np.asarray(p, dtype=np.float32)).reshape(B, H, W)
    t = np.ascontiguousarray(np.asarray(t, dtype=np.float32)).reshape(B, H, W)
    nc = _get_nc()
    bk = _banded_mats()
    p8 = p.astype(mybir.dt.np(FP8))
    t8 = t.astype(mybir.dt.np(FP8))
    in_maps = [
        {"p": p8[c * BPC:(c + 1) * BPC], "t": t8[c * BPC:(c + 1) * BPC],
         "bk": bk}
        for c in range(NCORES)
    ]
    res = run_bass_kernel_spmd(nc, in_maps, core_ids=list(range(NCORES)))
    # junk rows carried by the 128-partition tiles: 2 zero rows in each of
    # the 32 full band tiles + 64 zero rows in the packed tile; each
    # contributes softplus(0) = ln 2 at W columns.
    junk = (2 * BPC * NB + (128 - 8 * BPC)) * W * np.log(2.0)
    total = 0.0
    for c in range(NCORES):
        o = res.results[c]["out"].astype(np.float64)
        total += -o[:, 0].sum() - junk
    total += _edge_loss_sum(p, t) - _phantom_loss_sum(p, t)
    return np.float32(total / (B * H * W))


# revision 9
# speedup vs baseline: 1.1471x; 1.1401x over previous
"""Trainium2 Bass kernel for nn_BoundaryLoss (Sobel-boundary BCE loss).

loss = mean(softplus(z) - z*et) over B=64 images of 512x512, where
  ps  = sigmoid(p)
  z   = |conv(ps,GX)| + |conv(ps,GY)|          (SAME zero padding)
  et  = ((|conv(t,GX)| + |conv(t,GY)|) > 0)    (t binary)

Device strategy (8 cores, pure data parallel over batch):
  * p and t shipped as fp8e4m3 (t exact for 0/1); sigmoid output ps also
    fp8 so every conv matmul runs in fp8 DoubleRow mode (0.5 cyc/row).
  * |ex|+|ey| = max(|u|,|v|) with u = conv(ps, GX+GY), v = conv(ps, GX-GY)
    -- one abs_max tensor-tensor op straight out of PSUM replaces the
    abs-transit + add of the naive form.
  * Each of the three convs (u, v, wt = conv(t, GX+9GY)) is 2 DoubleRow
    matmuls: taps (j0,j1) paired, (j2, zero-row) paired.
  * Sign-folded softplus: loss_elem = -ln sigmoid(w), w = +z if et else -z.
    et = (wt != 0); the fold is pure bit math: m = (wt==0)<<15 (u16),
    w = z | m on the bf16 zstore.
  * Phase 2: sigmoid(w) -> 8-ary bf16 product tree -> one Ln pass with
    accum_out.  Exactly 2 ACT table loads (Sigmoid warm at t=0, Ln at end).
  * H split into 4 bands of 126 rows + one packed band (last 8 rows of all
    8 images block-diagonally, 72 partitions) -> no halo corrections.
  * DMA batched per image-pair (4 transfers per pair) to amortize the
    625ns/op HWDGE serialization; banded fp8 weight matrices shipped
    without their zero rows (memset on device).
  * Engines: PE 6 matmuls/unit; DVE abs_max + fold + product tree;
    Pool (gpsimd) the (wt==0) mask + 1/3 of abs_max; ACT sigmoids + Ln.
  * Device covers image cols 1..511 (+ phantom col 512 from the zero pad);
    host adds the w=0 column, subtracts the phantom contribution, and
    subtracts softplus(0)=ln2 for the structurally-zero junk rows.
"""

import os
import sys

import numpy as np

for _p in ("/opt/trn_rl_repo", os.path.expanduser("~/.axon_site/_ro/trn_rl_repo")):
    if os.path.isdir(_p) and _p not in sys.path:
        sys.path.append(_p)

import concourse.bass as bass
import concourse.bacc as bacc
import concourse.tile as tile
from concourse import mybir
from concourse.bass import _add_dep_helper
from concourse.bass_utils import run_bass_kernel_spmd

F32 = mybir.dt.float32
BF16 = mybir.dt.bfloat16
U32 = mybir.dt.uint32
U16 = mybir.dt.uint16
FP8 = mybir.dt.float8e4
U8 = mybir.dt.uint8
A = mybir.AluOpType
AF = mybir.ActivationFunctionType

NCORES = 8
B, H, W = 64, 512, 512
BPC = B // NCORES          # images per core
NB = 4                     # full 126-row bands per image
BAND = 126
NBP = BPC * NB + 1         # units per core (33)
WP = W + 4                 # padded tile width for shifted DoubleRow reads

# 3x3 kernels and the vertical-tap sets
_GX = np.array([[1., 0., -1.], [2., 0., -2.], [1., 0., -1.]])
_GY = np.array([[1., 2., 1.], [0., 0., 0.], [-1., -2., -1.]])
_CU = _GX + _GY            # u-conv:  |ex|+|ey| = max(|u|,|v|)
_CV = _GX - _GY
_CW = _GX + 9.0 * _GY      # wt-conv: et = (wt != 0), exact ints in fp8
# DVE-path units conv (U, V) and reduce with abs; ACT-path units conv
# (GX, GY) so the combine after ACT-Abs is an ADD (the only elementwise
# reduction Pool's ISA has).
_CONVS = [_CU, _CV, _CW, _GX, _GY]
NCV = len(_CONVS)


def _banded(tap, var):
    """[128,128] banded vertical-conv matrix for one tap and variant."""
    m = np.zeros((128, 128), np.float32)
    if var < 2:
        for q in range(BAND):
            for dh in range(3):
                p = q + dh - (1 if var == 1 else 0)
                if 0 <= p < 128:
                    m[p, q] = tap[dh]
    else:
        for j in range(BPC):
            for qq in range(8):
                for dh in range(3):
                    pp = qq + dh
                    if pp < 9:
                        m[9 * j + pp, 8 * j + qq] = tap[dh]
    return m


def _banded_mats():
    """[128, 3*NCV*4, 128] fp8: var(3) x conv(NCV) x (t0,t1,t2,Z) lhsT.
    The Z rows stay zero (DoubleRow second weight row for the lone tap);
    shipping them keeps the DMA a single contiguous descriptor/partition."""
    out = np.zeros((128, 3 * NCV * 4, 128), np.float32)
    for var in range(3):
        for ci, cm in enumerate(_CONVS):
            for k in range(3):
                out[:, (var * NCV + ci) * 4 + k, :] = _banded(cm[:, k], var)
    return out.astype(mybir.dt.np(FP8))


def _build_program(opts=()):
    opts = set(opts)
    nc = bacc.Bacc("TRN2", target_bir_lowering=False)
    p_d = nc.dram_tensor("p", [BPC, H, W], FP8, kind="ExternalInput")
    t_d = nc.dram_tensor("t", [BPC, H, W], FP8, kind="ExternalInput")
    bk_d = nc.dram_tensor("bk", [128, 3 * NCV * 4, 128], FP8, kind="ExternalInput")
    out_d = nc.dram_tensor("out", [128, 2], F32, kind="ExternalOutput")
    if "debug" in opts:
        dbg_z = nc.dram_tensor("dbg_z", [128, NBP, W], BF16, kind="ExternalOutput")
        dbg_q = nc.dram_tensor("dbg_q", [128, NBP, W], BF16, kind="ExternalOutput")

    DR = mybir.MatmulPerfMode.DoubleRow

    with tile.TileContext(nc) as tc:
        with tc.tile_pool(name="consts", bufs=1) as consts, \
             tc.tile_pool(name="xin", bufs=1) as xin, \
             tc.tile_pool(name="tin", bufs=1) as tin, \
             tc.tile_pool(name="psg", bufs=1) as psg, \
             tc.tile_pool(name="packed", bufs=1) as packed, \
             tc.tile_pool(name="mm", bufs=2) as mmp, \
             tc.tile_pool(name="ax", bufs=2) as axp, \
             tc.tile_pool(name="zs", bufs=1) as zs, \
             tc.tile_pool(name="accp", bufs=1) as accp, \
             tc.tile_pool(name="psum", bufs=2, space="PSUM") as psum, \
             tc.tile_pool(name="psum2", bufs=2, space="PSUM") as psum2:

            bk = consts.tile([128, 3 * NCV * 4, 128], FP8)  # var x conv x (t0,t1,t2,Z)

            warm = accp.tile([1, 1], F32)
            nc.gpsimd.memset(warm[:, :], 0.0)
            nc.scalar.activation(out=warm[:, :], in_=warm[:, :],
                                 func=AF.Sigmoid)

            x_all = xin.tile([128, BPC, NB, W], FP8)
            t_all = tin.tile([128, BPC, NB, WP], FP8)
            ps_all = psg.tile([128, BPC, NB, WP], FP8)
            x4 = packed.tile([72, W], FP8)
            t4 = packed.tile([72, WP], FP8)
            ps4 = packed.tile([72, WP], FP8)
            zstore = zs.tile([128, NBP, W], BF16)
            sq = zs.tile([128, NBP, W], BF16)
            acc_s = accp.tile([128, 1], F32)
            out_t = accp.tile([128, 2], F32)

            # zero right-pads (read by shifted DoubleRow taps), Z weight
            # rows, and the output accumulators
            nc.gpsimd.memset(t_all[:, :, :, W:WP].bitcast(U32), 0)
            nc.gpsimd.memset(ps_all[:, :, :, W:WP].bitcast(U32), 0)
            nc.gpsimd.memset(t4[:, W:WP].bitcast(U32), 0)
            nc.gpsimd.memset(ps4[:, W:WP].bitcast(U32), 0)
            nc.gpsimd.memset(out_t[:, :], 0.0)

            # ---- input DMA: one transfer for all band-0 slabs, one per
            # image for bands 1-3 (DMA APs are limited to 3 dims) ----
            def b0_dma(dram, dst):
                nc.sync.dma_start(
                    out=dst[:, :, 0, 0:W],
                    in_=bass.AP(tensor=dram[:, :, :].tensor, offset=0,
                                ap=[[W, 128], [H * W, BPC], [1, W]]))

            def b123_dma(dram, dst, i):
                nc.sync.dma_start(
                    out=dst[:, i, 1:4, 0:W],
                    in_=bass.AP(tensor=dram[:, :, :].tensor,
                                offset=i * H * W + (BAND - 1) * W,
                                ap=[[W, 128], [BAND * W, 3], [1, W]]))

            b0_dma(p_d, x_all)
            b123_dma(p_d, x_all, 0)
            b123_dma(p_d, x_all, 1)
            # banded weights (tap rows only; Z rows memset above)
            nc.sync.dma_start(
                out=bass.AP(tensor=bk.tensor, offset=bk.offset,
                            ap=[[bk.ap[0][0], 128], [4 * 128, 3 * NCV],
                                [128, 3], [1, 128]]),
                in_=bass.AP(tensor=bk_d[:, :, :].tensor, offset=0,
                            ap=[[3 * NCV * 3 * 128, 128],
                                [3 * 128, 3 * NCV],
                                [128, 3], [1, 128]]))
            b0_dma(t_d, t_all)
            b123_dma(t_d, t_all, 0)
            b123_dma(t_d, t_all, 1)
            for i in range(2, BPC):
                b123_dma(p_d, x_all, i)
                b123_dma(t_d, t_all, i)
            # packed band: last 9 rows of each image, partitions 9j+q = 0..71
            src9 = lambda dram: bass.AP(
                tensor=dram[:, :, :].tensor, offset=(H - 9) * W,
                ap=[[H * W, 8], [W, 9], [1, W]])
            nc.sync.dma_start(out=x4[0:72, :], in_=src9(p_d))
            nc.sync.dma_start(out=t4[0:72, 0:W], in_=src9(t_d))

            # ---- unit list ----
            units = []      # (vb, kk, ps_view, t_view)
            for i in range(BPC):
                for b in range(NB):
                    vb = 1 if b == 0 else 0
                    units.append((vb, 128, ps_all[:, i, b, :], t_all[:, i, b, :]))
            units.append((2, 72, ps4[:, :], t4[:, :]))

            def drmm(outap, kk, row, rhs_view, col, start, stop):
                rhs = bass.AP(tensor=rhs_view.tensor,
                              offset=rhs_view.offset + col,
                              ap=[[rhs_view.ap[0][0], kk], [1, 2], [1, W]])
                nc.tensor.matmul(outap, bk[0:kk, row:row + 2, :], rhs,
                                 start=start, stop=stop, perf_mode=DR)

            AX = mybir.AxisListType
            sig_w = []
            wt_pair = None
            s1 = None
            for u, (vb, kk, psv, tv) in enumerate(units):
                if u % 8 == 0:
                    # sigmoid for the image pair feeding units u..u+7
                    g = u // 8
                    if g < 4:
                        nc.scalar.activation(
                            out=ps_all[:, 2 * g:2 * g + 2, :, 0:W],
                            in_=x_all[:, 2 * g:2 * g + 2, :, :],
                            func=AF.Sigmoid)
                    else:
                        nc.scalar.activation(out=ps4[:, 0:W], in_=x4[:, :],
                                             func=AF.Sigmoid)

                P = psum.tile([128, 2, W], F32, tag="uv")
                half = u % 2
                if half == 0:
                    wt_pair = psum2.tile([128, 2, W], F32, tag="wt")
                    s1 = mmp.tile([128, 2, W], BF16, tag="s1")
                act_path = (u % 2 == 0) and u != NBP - 1
                base = vb * NCV * 4
                c0 = base + (12 if act_path else 0)   # GX/GY or U/V rows
                drmm(P[:, 0, :], kk, c0 + 0, psv, 0, True, False)
                drmm(P[:, 0, :], kk, c0 + 2, psv, 2, False, True)
                drmm(P[:, 1, :], kk, c0 + 4, psv, 0, True, False)
                drmm(P[:, 1, :], kk, c0 + 6, psv, 2, False, True)
                drmm(wt_pair[:, half, :], kk, base + 8, tv, 0, True, False)
                drmm(wt_pair[:, half, :], kk, base + 10, tv, 2, False, True)

                # z = |ex|+|ey|: ~4/9 of units via ACT Abs(ex,ey) + DVE add;
                # the rest conv (u,v) and z = max(|u|,|v|) as one DVE
                # abs-max reduce over the (u,v) axis
                if act_path:
                    ax = axp.tile([128, 2, W], BF16, tag="ax")
                    nc.scalar.activation(out=ax[:, :, :], in_=P[:, :, :],
                                         func=AF.Abs)
                    nc.vector.tensor_tensor(
                        out=zstore[:, u, :], in0=ax[:, 0, :], in1=ax[:, 1, :],
                        op=A.add)
                else:
                    nc.vector.tensor_reduce(
                        out=zstore[:, u, :],
                        in_=P.rearrange("p c w -> p w c"), axis=AX.X,
                        op=A.max, apply_absolute_value=True)

                if half == 1 or u == NBP - 1:
                    u0 = u - half
                    # s1 = (wt != 0) - 0.5 in {-0.5, +0.5}
                    nc.vector.tensor_scalar(
                        out=s1[:, 0:half + 1, :],
                        in0=wt_pair[:, 0:half + 1, :],
                        scalar1=0.0, scalar2=0.5,
                        op0=A.not_equal, op1=A.subtract)
                    # q' = z * s1 = +-z/2 (sigmoid below uses scale=2)
                    q_eng = nc.vector if (u // 2) % 4 < 3 else nc.gpsimd
                    q_eng.tensor_tensor(
                        out=zstore[:, u0:u + 1, :],
                        in0=zstore[:, u0:u + 1, :],
                        in1=s1[:, 0:half + 1, :], op=A.mult)

                # sigmoid(2*q') in chunks of 4 units
                if (u + 1) % 4 == 0 or u == NBP - 1:
                    lo = u // 4 * 4
                    s2 = nc.scalar.activation(
                        out=sq[:, lo:u + 1, :], in_=zstore[:, lo:u + 1, :],
                        func=AF.Sigmoid, scale=2.0)
                    sig_w.append(s2)

                # 8-ary product tree per completed group of 8
                if (u + 1) % 8 == 0:
                    g0 = u - 7
                    p_eng = nc.gpsimd
                    p_eng.tensor_tensor(
                        out=sq[:, g0:g0 + 8:2, :], in0=sq[:, g0:g0 + 8:2, :],
                        in1=sq[:, g0 + 1:g0 + 8:2, :], op=A.mult)
                    p_eng.tensor_tensor(
                        out=sq[:, g0:g0 + 8:4, :], in0=sq[:, g0:g0 + 8:4, :],
                        in1=sq[:, g0 + 2:g0 + 8:4, :], op=A.mult)
                    p_eng.tensor_tensor(
                        out=sq[:, g0, :], in0=sq[:, g0, :],
                        in1=sq[:, g0 + 4, :], op=A.mult)

            if "debug" in opts:
                nc.sync.dma_start(out=dbg_z[:, :, :], in_=zstore[:, :, :])
                nc.sync.dma_start(out=dbg_q[:, :, :], in_=sq[:, :, :])

            # ---- finale: one Ln pass over the 5 product slots ----
            li = nc.scalar.activation(
                out=zstore[:, 0:NBP:8, :], in_=sq[:, 0:NBP:8, :],
                func=AF.Ln, accum_out=acc_s[:, 0:1])
            _add_dep_helper(li.ins, sig_w[-1].ins, sync=True,
                            reason="ACT table phase split")
            nc.vector.tensor_copy(out=out_t[:, 0:1], in_=acc_s[:, 0:1])
            nc.sync.dma_start(out=out_d[:, :], in_=out_t)

    nc.compile()
    return nc


_NC_CACHE = None


def _get_nc():
    global _NC_CACHE
    if _NC_CACHE is None:
        _NC_CACHE = _build_program()
    return _NC_CACHE


def _edge_loss_sum(p, t):
    """float64 loss sum over the w=0 column (not computed on device)."""
    ps = 1.0 / (1.0 + np.exp(-p.astype(np.float64)))
    td = t.astype(np.float64)

    def slab(x):
        s = np.zeros((B, H + 2, 3))
        s[:, 1:H + 1, 1:3] = x[:, :, 0:2]
        return s

    sp, st = slab(ps), slab(td)

    def conv(x, K):
        acc = np.zeros((B, H))
        for dh in range(3):
            for dw in range(3):
                acc += K[dh, dw] * x[:, dh:dh + H, dw]
        return acc

    z = np.abs(conv(sp, _GX)) + np.abs(conv(sp, _GY))
    et = (np.abs(conv(st, _GX)) + np.abs(conv(st, _GY))) > 0
    return (np.logaddexp(0.0, z) - z * et).sum()


def _phantom_loss_sum(p, t):
    """float64 loss sum the device adds for its phantom column (image col
    512, fed by image col 511 + zero pads); subtracted from the total."""
    ps_col = 1.0 / (1.0 + np.exp(-p[:, :, W - 1].astype(np.float64)))
    t_col = t[:, :, W - 1].astype(np.float64)

    def vconv(col, tap):
        s = np.zeros((B, H + 2))
        s[:, 1:H + 1] = col
        return tap[0] * s[:, 0:H] + tap[1] * s[:, 1:H + 1] + tap[2] * s[:, 2:H + 2]

    ex = vconv(ps_col, np.array([1.0, 2.0, 1.0]))
    ey = vconv(ps_col, np.array([1.0, 0.0, -1.0]))
    wt = vconv(t_col, np.array([10.0, 2.0, -8.0]))
    z = np.abs(ex) + np.abs(ey)
    et = wt != 0
    return (np.logaddexp(0.0, z) - z * et).sum()


def kernel(p: np.ndarray, t: np.ndarray) -> np.ndarray:
    p = np.ascontiguousarray(np.asarray(p, dtype=np.float32)).reshape(B, H, W)
    t = np.ascontiguousarray(np.asarray(t, dtype=np.float32)).reshape(B, H, W)
    nc = _get_nc()
    bk = _banded_mats()
    p8 = p.astype(mybir.dt.np(FP8))
    t8 = t.astype(mybir.dt.np(FP8))
    in_maps = [
        {"p": p8[c * BPC:(c + 1) * BPC], "t": t8[c * BPC:(c + 1) * BPC],
         "bk": bk}
        for c in range(NCORES)
    ]
    res = run_bass_kernel_spmd(nc, in_maps, core_ids=list(range(NCORES)))
    # junk rows carried by the 128-partition tiles: 2 zero rows in each of
    # the 32 full band tiles + 64 zero rows in the packed tile; each
    # contributes softplus(0) = ln 2 at W columns.
    junk = (2 * BPC * NB + (128 - 8 * BPC)) * W * np.log(2.0)
    total = 0.0
    for c in range(NCORES):
        o = res.results[c]["out"].astype(np.float64)
        total += -o[:, 0].sum() - junk
    total += _edge_loss_sum(p, t) - _phantom_loss_sum(p, t)
    return np.float32(total / (B * H * W))
